# revision 20
# baseline (speedup 1.0000x reference)
"""CCLoss (Pearson correlation loss) Trainium2 kernel, 8-way data parallel.

Problem: y_pred ~ (64,1,480,640) f32, y_true ~ (64,1,480,640) f32.
reference: per-sample z-score (ddof=1) over (1,480,640), r = corr-like ratio,
loss = -mean(r).

Strategy: shard batch (64) across 8 cores, 8 samples/core. Each core computes
per-sample, per-partition moment partials in a single pass over the data
(memory-bound regime, HBM ~19.7MB/core at ~400GB/s is the bottleneck):
  - ScalarE (ACT):  sum(y^2) and sum(y) via activation accum_out (two passes)
  - VectorE (DVE):  sum(x*y) via scalar_tensor_tensor accum (the dedicated
                    tensor_tensor_reduce op crashes TRN2 here); mean/var of x
                    via bn_stats/bn_aggr (one pass in 480-wide chunks)
Partials accumulate into two engine-local tiles (one written only by DVE,
one only by ACT - cross-engine writes to one tile serialize under Tile's
coarse sub-tile dep tracking), DMA'd out as two tensors at the end. The last
sample's y is loaded in two halves so its ScalarE passes start on the first
half while the second still streams, shrinking the tail.
Partition-axis reduction and the final scalar math run on host in float64.
"""
import os
import sys

import numpy as np

for _p in ("/opt/trn_rl_repo", "/root/.axon_site/_ro/trn_rl_repo"):
    if os.path.isdir(_p) and _p not in sys.path:
        sys.path.append(_p)

import concourse.bass as bass
import concourse.mybir as mybir
import concourse.tile as tile
from concourse import bacc
from concourse.bass_utils import run_bass_kernel_spmd

NCORES = 8
B = 64
SPB = B // NCORES          # samples per core
P = 128                    # SBUF partitions
N = 1 * 480 * 640          # elements per sample
F = N // P                 # free dim per partition (2400)
NCHUNK = 5                 # bn_stats hardware limit: <=512 elems per call
CW = F // NCHUNK           # 480
EPS = 1e-8

FP32 = mybir.dt.float32

_CACHE = {}
LAST_RESULTS = None


def _build():
    nc = bacc.Bacc("TRN2", target_bir_lowering=False, debug=False,
                   enable_asserts=False)
    yp_d = nc.dram_tensor("yp", (SPB, P, F), FP32, kind="ExternalInput").ap()
    yt_d = nc.dram_tensor("yt", (SPB, P, F), FP32, kind="ExternalInput").ap()
    # per-partition partials, engine-local tiles -> two output tensors.
    # The last sample's y-dependent sums are split into two half-accumulators
    # (columns SPB-1 and SPB) so its ACT/DVE work can start on the first half
    # while the second half is still streaming in; host adds the two columns.
    # dve: [P, 2*SPB + (SPB+1)] = (mean_x, var_x) per sample + sxy columns
    # act: [P, 2*(SPB+1)] = syy columns + sy columns
    NYC = SPB + 1              # y-sum column count (last sample uses two)
    dve_d = nc.dram_tensor("dve", (P, 2 * SPB + NYC), FP32,
                           kind="ExternalOutput").ap()
    act_d = nc.dram_tensor("act", (P, 2 * NYC), FP32,
                           kind="ExternalOutput").ap()

    with tile.TileContext(nc) as tc:
        with (
            tc.tile_pool(name="data", bufs=7) as data,
            tc.tile_pool(name="scratch", bufs=3) as scratch,
            tc.tile_pool(name="stats", bufs=2) as stats,
            tc.tile_pool(name="persist", bufs=1) as persist,
        ):
            st_dve = persist.tile([P, 2 * SPB + NYC], FP32)
            st_act = persist.tile([P, 2 * NYC], FP32)
            nc.vector.memset(st_dve[:], 0.0)
            nc.vector.memset(st_act[:], 0.0)

            def y_sums(xt, ypart, syy_col, sy_col, sxy_col, xlo, xhi):
                sq = scratch.tile([P, xhi - xlo], FP32, tag="junk",
                                  name=f"sq{syy_col}")
                nc.scalar.activation(
                    sq[:], ypart, mybir.ActivationFunctionType.Square,
                    accum_out=st_act[:, syy_col:syy_col + 1],
                )
                cpy = scratch.tile([P, xhi - xlo], FP32, tag="junk",
                                   name=f"cpy{syy_col}")
                nc.scalar.activation(
                    cpy[:], ypart, mybir.ActivationFunctionType.Copy,
                    accum_out=st_act[:, NYC + sy_col:NYC + sy_col + 1],
                )
                prod = scratch.tile([P, xhi - xlo], FP32, tag="junk",
                                    name=f"prod{syy_col}")
                nc.vector.scalar_tensor_tensor(
                    out=prod[:], in0=xt[:, xlo:xhi], scalar=1.0, in1=ypart,
                    op0=mybir.AluOpType.mult, op1=mybir.AluOpType.mult,
                    accum_out=st_dve[:, 2 * SPB + sxy_col:2 * SPB + sxy_col + 1],
                )

            H1 = F // 2   # last-sample y split point
            for s in range(SPB):
                xt = data.tile([P, F], FP32)
                nc.sync.dma_start(xt[:], yp_d[s])
                last = s == SPB - 1
                if not last:
                    yt = data.tile([P, F], FP32)
                    nc.sync.dma_start(yt[:], yt_d[s])
                else:
                    yta = data.tile([P, H1], FP32, tag="yhalfa", bufs=2)
                    nc.sync.dma_start(yta[:], yt_d[s, :, 0:H1])
                    ytb = data.tile([P, F - H1], FP32, tag="yhalfb", bufs=2)
                    nc.sync.dma_start(ytb[:], yt_d[s, :, H1:F])

                # VectorE: mean/var of x per partition
                st6 = stats.tile([P, NCHUNK, 6], FP32)
                for c in range(NCHUNK):
                    nc.vector.bn_stats(st6[:, c, :], xt[:, c * CW:(c + 1) * CW])
                nc.vector.bn_aggr(st_dve[:, 2 * s:2 * s + 2], st6[:])

                # ScalarE: sum(y^2), sum(y); VectorE: sum(x*y)
                if not last:
                    y_sums(xt, yt[:], s, s, s, 0, F)
                else:
                    y_sums(xt, yta[:], s, s, s, 0, H1)
                    y_sums(xt, ytb[:], s + 1, s + 1, s + 1, H1, F)

            nc.sync.dma_start(dve_d[:], st_dve[:])
            nc.scalar.dma_start(act_d[:], st_act[:])

    nc.compile()
    return nc


def _get_nc():
    if "nc" not in _CACHE:
        _CACHE["nc"] = _build()
    return _CACHE["nc"]


def kernel(y_pred: np.ndarray, y_true: np.ndarray) -> np.ndarray:
    global LAST_RESULTS
    nc = _get_nc()

    yp = np.ascontiguousarray(np.asarray(y_pred, dtype=np.float32).reshape(B, P, F))
    yt = np.ascontiguousarray(np.asarray(y_true, dtype=np.float32).reshape(B, P, F))

    in_maps = [
        {"yp": yp[c * SPB:(c + 1) * SPB], "yt": yt[c * SPB:(c + 1) * SPB]}
        for c in range(NCORES)
    ]
    trace = bool(os.environ.get("CCLOSS_TRACE"))
    try:
        res = run_bass_kernel_spmd(nc, in_maps, core_ids=list(range(NCORES)),
                                   trace=trace)
    except Exception:
        if not trace:
            raise
        res = run_bass_kernel_spmd(nc, in_maps, core_ids=list(range(NCORES)),
                                   trace=False)
    LAST_RESULTS = res

    r_all = np.empty(B, dtype=np.float64)
    n = float(N)
    for c in range(NCORES):
        NYC = SPB + 1
        dv = res.results[c]["dve"].astype(np.float64)   # [P, 2*SPB+NYC]
        ac = res.results[c]["act"].astype(np.float64)   # [P, 2*NYC]
        for s in range(SPB):
            mean_p = dv[:, 2 * s]
            var_p = dv[:, 2 * s + 1]
            Sx = F * mean_p.sum()
            Sxx = F * (var_p + mean_p * mean_p).sum()
            last = s == SPB - 1
            cols = (s, s + 1) if last else (s,)
            Sxy = sum(dv[:, 2 * SPB + t].sum() for t in cols)
            Syy = sum(ac[:, t].sum() for t in cols)
            Sy = sum(ac[:, NYC + t].sum() for t in cols)

            cxx = Sxx - Sx * Sx / n            # sum((x-mu_x)^2)
            cyy = Syy - Sy * Sy / n
            cxy = Sxy - Sx * Sy / n
            sdx = np.sqrt(cxx / (n - 1.0)) + EPS
            sdy = np.sqrt(cyy / (n - 1.0)) + EPS

            num = cxy / (sdx * sdy)            # sum(a*b)
            saa = cxx / (sdx * sdx)            # sum(a*a)
            sbb = cyy / (sdy * sdy)
            r = num / np.sqrt(saa * sbb + EPS)
            r_all[c * SPB + s] = r

    loss = -r_all.mean()
    return np.array(loss, dtype=np.float32)
